# revision 12
# baseline (speedup 1.0000x reference)
"""Single-head attention (B=4, N=4096, E=1024, H=64) on 8 TRN2 NeuronCores.

Sharding v4: core c = (batch b = c//2, half h = c%2). Each core projects
k/q/v ONLY for its own 2048 rows (half the x traffic and half the projection
matmuls of v2), then the core pair exchanges K/V via an HBM AllGather:

  own kvT (chunks 0:15) -> kvx (DRAM) -> AllGather pair -> kvg[2,...]
  -> dynamic-offset DMA kvg[1 - (core_id & 1)] -> kvT_partner (chunks 16:31)

The partner index is a runtime register (PartitionIdOp), so the program stays
pure SPMD. Attention over own chunks rides during the x stream; partner
chunks follow once the exchange lands (it hides under own-chunk attention).

Attention groups are qb-PAIRED: one S matmul per key chunk streams BOTH query
blocks of a pair (F=1024 bf16), halving LDWEIGHTS traffic and S instruction
count; exp runs on one [128,1024] tile; two PV matmuls (F=512) accumulate the
pair's O tiles with the fused ones-column denominators. PV trails by 2 groups
so the PE never waits on exp. exp is split: ScalarE exact / DVE Schraudolph
(int16 bit trick, ~1.8% rms) on a minority of groups.

Projection stationaries as in v2: [Wk|Wv] even n-subchunks, [Wv|Wk] odd
(parity keeps k-even/v-odd in partitions 0:64 etc.), [Wq|Wq] fills both
PSUM halves so S row groups read q without cross-duplication.

PSUM: kv 1 + q/transpose 1 + 2x2 S + 2 O banks. Host assembles out = outT.T.
"""

import math
import tempfile

import ml_dtypes
import numpy as np

import concourse.bass as bass
import concourse.tile as tile
from concourse import bacc, mybir
from concourse.bass_utils import run_bass_kernel_spmd
from concourse.masks import make_identity

B, N, E, H = 4, 4096, 1024, 64
NCORES = 8
NQ = N // 2  # rows owned per core (queries; also its share of keys)
QB = 512  # query block
NKC = N // 128  # 32 key chunks of 128 (16 own + 16 partner)
OWNC = NKC // 2
ECH = E // 128  # 8 embedding chunks
NBLK = NQ // QB  # 4 x blocks per core
QBLKS = NQ // QB  # 4 query blocks per core
WCOLS = 3 * 128  # [Wk|Wv], [Wv|Wk], [Wq|Wq]

F32 = mybir.dt.float32
BF16 = mybir.dt.bfloat16

SCALE = 1.0 / np.sqrt(H)
# Schraudolph fast-exp constants (DVE path): int16(s*A + B) bits -> bfloat16
SCHRA_A = 128.0 * SCALE / math.log(2.0)
SCHRA_B = 16248.6

PAIRS = [[0, 1], [2, 3], [4, 5], [6, 7]]


def build_kernel():
    nc = bacc.Bacc("TRN2", target_bir_lowering=False, debug=False, num_devices=NCORES)

    xT_d = nc.dram_tensor("xT", [E, NQ], BF16, kind="ExternalInput")
    wT_d = nc.dram_tensor("wT", [E, WCOLS], BF16, kind="ExternalInput")
    outT_d = nc.dram_tensor("outT", [H, NQ], F32, kind="ExternalOutput")
    sums_d = nc.dram_tensor("sums_bounce", [2, QB], F32)
    kvx_d = nc.dram_tensor("kvx", [128, NQ], BF16)
    kvg_d = nc.dram_tensor("kvg", [2, 128, NQ], BF16)

    xT = xT_d.ap().rearrange("(c p) n -> p c n", p=128)  # [128, ECH, NQ]
    wT = wT_d.ap().rearrange("(c p) h -> p c h", p=128)
    outT = outT_d.ap()
    sums = sums_d.ap()

    with tile.TileContext(nc) as tc:
        with (
            tc.tile_pool(name="singles", bufs=1) as singles,
            tc.tile_pool(name="xpool", bufs=NBLK) as xpool,
            tc.tile_pool(name="qkv", bufs=1) as qkv,
            tc.tile_pool(name="ppool", bufs=5) as ppool,
            tc.tile_pool(name="npool", bufs=2) as npool,
            tc.tile_pool(name="kv_ps", bufs=1, space="PSUM") as kv_pool,
            tc.tile_pool(name="qtr_ps", bufs=1, space="PSUM") as qtr_pool,
            tc.tile_pool(name="s_ps", bufs=2, space="PSUM") as s_pool,
            tc.tile_pool(name="o_ps", bufs=2, space="PSUM") as o_pool,
        ):
            wsb = singles.tile([128, ECH, WCOLS], BF16)
            nc.sync.dma_start(out=wsb[:], in_=wT)
            x_t0 = xpool.tile([128, ECH, QB], BF16, name="x_t")
            for piece in range(4):
                nc.sync.dma_start(
                    out=x_t0[:, 2 * piece : 2 * piece + 2, :],
                    in_=xT[:, 2 * piece : 2 * piece + 2, 0:QB],
                )
            ident = singles.tile([128, H], BF16)
            make_identity(nc, ident[0:H, :])
            nc.scalar.dma_start(out=ident[H : 2 * H, :], in_=ident[0:H, :])

            # own-half kv (chunks 0:15) and partner-half kv (chunks 16:31,
            # landed by the exchange). Separate tiles so the partner DMA
            # can never false-serialize against own-chunk attention.
            kvT_own = qkv.tile([128, NQ], BF16)
            kvT_par = qkv.tile([128, NQ], BF16)
            qT_sb = qkv.tile([128, NQ], BF16)
            v_all = qkv.tile([128, NKC, 80], BF16)
            nc.vector.memset(v_all[:, :, H : H + 1], 1.0)

            ones_h = singles.tile([1, H], F32)
            nc.vector.memset(ones_h[:], 1.0)

            junk = singles.tile([128, 256], BF16)
            nc.vector.memset(junk[:], 0.5)
            warm_ps = kv_pool.tile([128, QB], F32, name="warm_ps", tag="kv")
            for _ in range(12):
                nc.tensor.matmul(
                    warm_ps[0:H, 0:192], junk[:, 0:H], junk[:, 64:256],
                    start=True, stop=True, tile_position=(0, 0),
                )

            def kv_chunk(c, r=slice(0, 128)):
                # chunk slot c -> AP slice [r, chunk cols]; parity layout inside
                t = kvT_own if c < OWNC else kvT_par
                cc = c if c < OWNC else c - OWNC
                return t[r, cc * 128 : (cc + 1) * 128]

            def emit_tr(c):
                # V-natural tile for chunk c (even: v hi, odd: v lo)
                vlo = (c % 2) == 1
                r = slice(0, H) if vlo else slice(H, 2 * H)
                v_tr = qtr_pool.tile([128, H], BF16, name="v_tr", tag="qtr")
                nc.tensor.transpose(
                    v_tr[:], kv_chunk(c, r), ident[r, :],
                    tile_position=(0 if vlo else H, 0),
                )
                nc.vector.tensor_copy(v_all[:, c, 0:H], v_tr[:])

            # --- attention: qb-paired groups --------------------------------
            o_acc = [None] * QBLKS
            pv_lag = {}  # pair -> list of (c, p_t)
            next_c = {0: 0, 1: 0}  # pair index -> next chunk
            exp_cnt = [0]

            def emit_pv(pair, c, p_t):
                qa, qb_ = 2 * pair, 2 * pair + 1
                for j, q in enumerate((qa, qb_)):
                    nc.tensor.matmul(
                        o_acc[q][:],
                        v_all[:, c, 0 : H + 1],
                        p_t[:, j * QB : (j + 1) * QB],
                        start=(c == 0), stop=(c == NKC - 1),
                    )

            def emit_group(pair, c, phase=0):
                for q in (2 * pair, 2 * pair + 1):
                    if o_acc[q] is None:
                        o_acc[q] = o_pool.tile(
                            [H + 1, QB], F32, name=f"o_qb{q}", tag="o_acc"
                        )
                if c >= OWNC:
                    emit_tr_maybe(c)
                lo = c % 2 == 0
                r = slice(0, H) if lo else slice(H, 2 * H)
                qsl = slice(pair * 2 * QB, (pair + 1) * 2 * QB)
                # two F=512 matmuls (F=1024 fails the ISA check); same
                # stationary back-to-back so the second needs no fresh load
                s_t = s_pool.tile([128, 2 * QB], F32, name="s_t")
                for j in range(2):
                    nc.tensor.matmul(
                        s_t[:, j * QB : (j + 1) * QB],
                        kv_chunk(c, r),
                        qT_sb[r, qsl.start + j * QB : qsl.start + (j + 1) * QB],
                        start=True, stop=True,
                        tile_position=(0 if lo else H, 0),
                    )
                p_t = ppool.tile([128, 2 * QB], BF16, name="p_t")
                c_ = exp_cnt[0]
                exp_cnt[0] += 1
                use_dve = (c_ % 8 in (3, 6)) if phase == 0 else (c_ % 8 in (1, 3, 6))
                if use_dve:
                    nc.vector.tensor_scalar(
                        out=p_t[:].bitcast(mybir.dt.int16), in0=s_t[:],
                        scalar1=SCHRA_A, scalar2=SCHRA_B,
                        op0=mybir.AluOpType.mult, op1=mybir.AluOpType.add,
                    )
                else:
                    nc.scalar.activation(
                        p_t[:], s_t[:],
                        mybir.ActivationFunctionType.Exp, scale=SCALE,
                    )
                lag = pv_lag.setdefault(pair, [])
                lag.append((c, p_t))
                if len(lag) > 2:
                    emit_pv(pair, *lag.pop(0))

            tr_done = set()

            def emit_tr_maybe(c):
                if c not in tr_done:
                    tr_done.add(c)
                    emit_tr(c)

            def finish_qb(q, fast):
                o_t = o_acc[q]
                s_row = npool.tile([1, QB], F32, name="s_row")
                nc.vector.tensor_copy(s_row[:], o_t[H : H + 1, :])
                if fast:
                    rep_ps = s_pool.tile([H, QB], F32, name="rep_ps", tag="s_t")
                    nc.tensor.matmul(
                        rep_ps[:], ones_h[:], s_row[:], start=True, stop=True
                    )
                    s_rep = rep_ps
                else:
                    nc.sync.dma_start(out=sums[q : q + 1, :], in_=s_row[:])
                    s_rep = npool.tile([H, QB], F32, name="s_rep")
                    nc.sync.dma_start(
                        out=s_rep[:],
                        in_=bass.AP(
                            tensor=sums.tensor, offset=q * QB,
                            ap=[[0, H], [1, QB]],
                        ),
                    )
                r_rep = npool.tile([H, QB], F32, name="r_rep")
                nc.vector.reciprocal_approx_fast(out=r_rep[:], in_=s_rep[:])
                o_n = npool.tile([H, QB], F32, name="o_n")
                nc.vector.tensor_mul(o_n[:], o_t[0:H, :], r_rep[:])
                nc.sync.dma_start(out=outT[:, q * QB : (q + 1) * QB], in_=o_n[:])

            def finish_pair(pair, fast):
                lag = pv_lag[pair]
                while lag:
                    emit_pv(pair, *lag.pop(0))
                finish_qb(2 * pair, fast)
                finish_qb(2 * pair + 1, fast)

            # --- phase B: own-half projections (4 blocks) -------------------
            x_tiles = {0: x_t0}
            for nb in range(1, NBLK):
                x_t = xpool.tile([128, ECH, QB], BF16, name="x_t")
                nc.sync.dma_start(
                    out=x_t[:], in_=xT[:, :, nb * QB : (nb + 1) * QB]
                )
                x_tiles[nb] = x_t

            kvT4 = kvT_own.rearrange("p (c t) -> p c t", t=128)
            qT4 = qT_sb.rearrange("p (c t) -> p c t", t=128)
            for nb in range(NBLK):
                x_t = x_tiles.pop(nb)
                x4 = x_t.rearrange("p e (c t) -> p e c t", t=128)
                kv_ps = kv_pool.tile([128, QB], F32, name="kv_ps", tag="kv")
                for ec in range(ECH):
                    nc.tensor.matmul(
                        kv_ps[:, 0:256], wsb[:, ec, 0:128],
                        x4[:, ec, 0:4:2, :],
                        start=(ec == 0), stop=(ec == ECH - 1),
                    )
                cb = nb * 4
                nc.vector.tensor_copy(
                    kvT4[:, cb : cb + 4 : 2, :],
                    kv_ps[:, 0:256].rearrange("p (c t) -> p c t", t=128),
                )
                for ec in range(ECH):
                    nc.tensor.matmul(
                        kv_ps[:, 256:512], wsb[:, ec, 128:256],
                        x4[:, ec, 1:4:2, :],
                        start=(ec == 0), stop=(ec == ECH - 1),
                    )
                nc.vector.tensor_copy(
                    kvT4[:, cb + 1 : cb + 4 : 2, :],
                    kv_ps[:, 256:512].rearrange("p (c t) -> p c t", t=128),
                )
                q_ps = qtr_pool.tile([128, QB], F32, name="q_ps", tag="qtr")
                for ec in range(ECH):
                    nc.tensor.matmul(
                        q_ps[:, 0:256], wsb[:, ec, 256:384],
                        x4[:, ec, 0:4:2, :],
                        start=(ec == 0), stop=(ec == ECH - 1),
                    )
                nc.vector.tensor_copy(
                    qT4[:, cb : cb + 4 : 2, :],
                    q_ps[:, 0:256].rearrange("p (c t) -> p c t", t=128),
                )
                for ec in range(ECH):
                    nc.tensor.matmul(
                        q_ps[:, 256:512], wsb[:, ec, 256:384],
                        x4[:, ec, 1:4:2, :],
                        start=(ec == 0), stop=(ec == ECH - 1),
                    )
                nc.vector.tensor_copy(
                    qT4[:, cb + 1 : cb + 4 : 2, :],
                    q_ps[:, 256:512].rearrange("p (c t) -> p c t", t=128),
                )
                for j in range(4):
                    emit_tr_maybe(cb + j)
                # ride pair-0 groups on own chunks (needs qb0+qb1: nb >= 1)
                if nb >= 1:
                    avail = 4 * (nb + 1)
                    while next_c[0] < min(avail, OWNC):
                        emit_group(0, next_c[0], phase=0)
                        next_c[0] += 1

            # --- phase C: pair exchange of kv halves ------------------------
            nc.sync.dma_start(out=kvx_d.ap(), in_=kvT_own[:])
            nc.gpsimd.collective_compute(
                "AllGather", mybir.AluOpType.bypass,
                replica_groups=PAIRS,
                ins=[kvx_d.ap()], outs=[kvg_d.ap()],
            )
            partner = 1 - (nc.partition_id() & 1)
            nc.sync.dma_start(
                out=kvT_par[:],
                in_=kvg_d.ap()[bass.ds(partner, 1), :, :].rearrange("o p n -> (o p) n"),
            )

            # --- phases D-H: remaining attention ----------------------------
            while next_c[0] < OWNC:  # pair 0 own leftovers
                emit_group(0, next_c[0], phase=1)
                next_c[0] += 1
            while next_c[0] < NKC:  # pair 0 partner chunks
                emit_group(0, next_c[0], phase=1)
                next_c[0] += 1
            finish_pair(0, fast=False)
            while next_c[1] < NKC:  # pair 1: all 32 chunks
                emit_group(1, next_c[1], phase=1)
                next_c[1] += 1
            finish_pair(1, fast=True)

    nc.compile()
    return nc


_NC_CACHE = {}


def _get_nc():
    if "nc" not in _NC_CACHE:
        _NC_CACHE["nc"] = build_kernel()
    return _NC_CACHE["nc"]


def _make_in_maps(x, Wk, Wq, Wv):
    BF = ml_dtypes.bfloat16
    wkT, wqT, wvT = Wk.T, Wq.T, Wv.T
    wpack = np.ascontiguousarray(
        np.concatenate([wkT, wvT, wvT, wkT, wqT, wqT], axis=1), dtype=np.float32
    ).astype(BF)
    in_maps = []
    for c in range(NCORES):
        b, h = divmod(c, 2)
        xb = np.asarray(x[b][h * NQ : (h + 1) * NQ], dtype=np.float32)
        in_maps.append(
            {"xT": np.ascontiguousarray(xb.T).astype(BF), "wT": wpack}
        )
    return in_maps


def kernel(x, Wk, Wq, Wv, _trace=False, _tmpdir=None):
    nc = _get_nc()
    in_maps = _make_in_maps(x, Wk, Wq, Wv)
    kwargs = {}
    if _trace:
        kwargs = dict(trace=True, tmpdir=_tmpdir or tempfile.mkdtemp())
    res = run_bass_kernel_spmd(nc, in_maps, core_ids=list(range(NCORES)), **kwargs)
    out = np.empty((B, N, H), np.float32)
    for c in range(NCORES):
        b, h = divmod(c, 2)
        out[b, h * NQ : (h + 1) * NQ, :] = res.results[c]["outT"].T
    if _trace:
        return out, res
    return out


# revision 14
# speedup vs baseline: 1.6265x; 1.6265x over previous
"""Single-head attention (B=4, N=4096, E=1024, H=64) on 8 TRN2 NeuronCores.

Sharding: core c = (batch b = c//2, query-half h = c%2). Each core computes the
full K/V projections for its batch and attention for its 2048 query rows.
Attention is permutation-invariant over keys, so each core receives its batch's
x pre-transposed ([E, N], embedding on partitions) with its OWN query half in
columns 0:2048 - the program is identical across cores (pure SPMD), only the
data differs.

v2 (this file) vs the 146us baseline:
  - x and the packed weights are cast to bf16 on the HOST: x DMA halves
    (16 MiB -> 8 MiB, ~25us) and the on-chip fp32->bf16 casts (36us of DVE)
    disappear.
  - k/v projections use M=128 stationaries [Wk|Wv] (even n-subchunks) and
    [Wv|Wk] (odd n-subchunks): one x stream computes BOTH k and v, halving
    projection stream columns. The swapped odd stationary reproduces the
    baseline's parity layout (k-even/v-odd in partitions 0:64, k-odd/v-even
    in 64:128) so S row-group alternation and V-transpose placement are
    unchanged. k and v drain into ONE kvT tile with 2 copies per block.
  - q projection uses [Wq|Wq]: both PSUM partition halves get q, so the
    cross-duplication DMAs of the baseline are gone.
  - attention pipeline (S^T groups, exp on ScalarE, lagged PV with fused
    ones-column denominators, tail normalize) is unchanged.
PSUM: 1 kv bank + 1 q/transpose bank + 2x2-bank S groups + 2 O banks.
Host assembles out[b, half] = outT.T.
"""

import math
import tempfile

import ml_dtypes
import numpy as np

import concourse.bass as bass
import concourse.tile as tile
from concourse import bacc, mybir
from concourse.bass_utils import run_bass_kernel_spmd
from concourse.masks import make_identity

B, N, E, H = 4, 4096, 1024, 64
NCORES = 8
NQ = N // 2  # query rows per core
QB = 512  # query block (free dim of attention matmuls)
NKC = N // 128  # 32 key chunks of 128
ECH = E // 128  # 8 embedding chunks of 128
NB = N // QB  # 8 projection column blocks
QBLKS = NQ // QB  # 4 query blocks per core
GRP = 2  # key chunks per S/exp group (PSUM banks per S tile)
NGROUPS = NKC // GRP  # 16 S/exp groups per query block
WCOLS = 3 * 128  # packed weight columns: [Wk|Wv], [Wv|Wk], [Wq|Wq]

F32 = mybir.dt.float32
BF16 = mybir.dt.bfloat16

SCALE = 1.0 / np.sqrt(H)

# Schraudolph fast-exp constants for the DVE path: exp(s*SCALE) is
# approximated by writing int16(s*A + B) and reinterpreting the bits as
# bfloat16 (2^x via the exponent field, linear mantissa interpolation).
# RMS relative error 1.8%; only a minority of softmax tiles use it.
SCHRA_A = 128.0 * SCALE / math.log(2.0)
SCHRA_B = 16248.6


def build_kernel():
    nc = bacc.Bacc("TRN2", target_bir_lowering=False, debug=False, num_devices=NCORES)

    xT_d = nc.dram_tensor("xT", [E, N], BF16, kind="ExternalInput")
    wT_d = nc.dram_tensor("wT", [E, WCOLS], BF16, kind="ExternalInput")
    outT_d = nc.dram_tensor("outT", [H, NQ], F32, kind="ExternalOutput")
    sums_d = nc.dram_tensor("sums_bounce", [QBLKS, QB], F32)

    xT = xT_d.ap().rearrange("(c p) n -> p c n", p=128)  # [128, ECH, N]
    wT = wT_d.ap().rearrange("(c p) h -> p c h", p=128)  # [128, ECH, 384]
    outT = outT_d.ap()
    sums = sums_d.ap()

    with tile.TileContext(nc) as tc:
        with (
            tc.tile_pool(name="singles", bufs=1) as singles,
            tc.tile_pool(name="xpool", bufs=8) as xpool,
            tc.tile_pool(name="qkv", bufs=1) as qkv,
            tc.tile_pool(name="ppool", bufs=5) as ppool,
            tc.tile_pool(name="npool", bufs=2) as npool,
            tc.tile_pool(name="kv_ps", bufs=1, space="PSUM") as kv_pool,
            tc.tile_pool(name="qtr_ps", bufs=1, space="PSUM") as qtr_pool,
            tc.tile_pool(name="s_ps", bufs=2, space="PSUM") as s_pool,
            tc.tile_pool(name="o_ps", bufs=2, space="PSUM") as o_pool,
        ):
            # weights first (they gate the first projection matmuls), then x
            # block 0 in quarter-pieces so the first chain can start early
            wsb = singles.tile([128, ECH, WCOLS], BF16)
            nc.sync.dma_start(out=wsb[:], in_=wT)
            x_t0 = xpool.tile([128, ECH, QB], BF16, name="x_t")
            for piece in range(4):
                nc.sync.dma_start(
                    out=x_t0[:, 2 * piece : 2 * piece + 2, :],
                    in_=xT[:, 2 * piece : 2 * piece + 2, 0:QB],
                )
            # identity (both halves) for PE transposes of vT
            ident = singles.tile([128, H], BF16)
            make_identity(nc, ident[0:H, :])
            nc.scalar.dma_start(out=ident[H : 2 * H, :], in_=ident[0:H, :])

            # persistent activations. kvT layout per 128-col chunk:
            #   even chunk: k in partitions 0:64, v in 64:128
            #   odd  chunk: v in partitions 0:64, k in 64:128
            # (S row groups alternate; V transposes pick the matching half.)
            kvT_sb = qkv.tile([128, N], BF16)
            qT_sb = qkv.tile([128, NQ], BF16)
            # V-natural tiles with fused ones column (softmax denominators);
            # chunk stride padded to 80
            v_all = qkv.tile([128, NKC, 80], BF16)
            nc.vector.memset(v_all[:, :, H : H + 1], 1.0)

            ones_h = singles.tile([1, H], F32)
            nc.vector.memset(ones_h[:], 1.0)

            # PE warmup from t~0: junk matmuls on a memset tile (not gated
            # by the weight DMA) so the HAM clock gate opens before real work
            junk = singles.tile([128, 256], BF16)
            nc.vector.memset(junk[:], 0.5)
            warm_ps = kv_pool.tile([128, QB], F32, name="warm_ps", tag="kv")
            for _ in range(12):
                nc.tensor.matmul(
                    warm_ps[0:H, 0:192], junk[:, 0:H], junk[:, 64:256],
                    start=True, stop=True, tile_position=(0, 0),
                )

            o_acc = [None] * QBLKS
            next_chunk = [0] * QBLKS
            pv_lag = [[] for _ in range(QBLKS)]

            def s_matmul(s_slice, c, qsl):
                # even chunks live in partitions 0:64, odd chunks in 64:128
                lo = c % 2 == 0
                r = slice(0, H) if lo else slice(H, 2 * H)
                nc.tensor.matmul(
                    s_slice,
                    kvT_sb[r, c * 128 : (c + 1) * 128],
                    qT_sb[r, qsl],
                    start=True, stop=True,
                    tile_position=(0 if lo else H, 0),
                )

            def group_chunks(i):
                # adjacent chunks pair lo/hi row-groups within one x block
                return [2 * i, 2 * i + 1]

            def emit_pv(qb, chunks, p_t):
                for j, c in enumerate(chunks):
                    nc.tensor.matmul(
                        o_acc[qb][:],
                        v_all[:, c, 0 : H + 1],
                        p_t[:, j * QB : (j + 1) * QB],
                        start=(c == 0), stop=(c == NKC - 1),
                    )

            exp_cnt = [0]

            def emit_group(qb, i, phase=0):
                if o_acc[qb] is None:
                    o_acc[qb] = o_pool.tile(
                        [H + 1, QB], F32, name=f"o_qb{qb}", tag="o_acc"
                    )
                qsl = slice(qb * QB, (qb + 1) * QB)
                chunks = group_chunks(i)
                s_t = s_pool.tile([128, GRP * QB], F32, name="s_t")
                for j, c in enumerate(chunks):
                    s_matmul(s_t[:, j * QB : (j + 1) * QB], c, qsl)
                p_t = ppool.tile([128, GRP * QB], BF16, name="p_t")
                # split exp across engines: ScalarE runs exact exp; DVE takes
                # a minority of groups via the Schraudolph bit trick (during
                # the x stream DVE is busy with drains, so fewer land there)
                c_ = exp_cnt[0]
                exp_cnt[0] += 1
                use_dve = (c_ % 5 == 3) if phase == 0 else (c_ % 5 in (1, 3))
                if use_dve:
                    nc.vector.tensor_scalar(
                        out=p_t[:].bitcast(mybir.dt.int16), in0=s_t[:],
                        scalar1=SCHRA_A, scalar2=SCHRA_B,
                        op0=mybir.AluOpType.mult, op1=mybir.AluOpType.add,
                    )
                else:
                    nc.scalar.activation(
                        p_t[:], s_t[:],
                        mybir.ActivationFunctionType.Exp, scale=SCALE,
                    )
                pv_lag[qb].append((chunks, p_t))
                if len(pv_lag[qb]) > 2:  # PV trails by 2 groups so the PE
                    emit_pv(qb, *pv_lag[qb].pop(0))  # never waits on exp

            def finish_qb(qb, fast=False):
                while pv_lag[qb]:
                    emit_pv(qb, *pv_lag[qb].pop(0))
                o_t = o_acc[qb]
                s_row = npool.tile([1, QB], F32, name="s_row")
                nc.vector.tensor_copy(s_row[:], o_t[H : H + 1, :])
                if fast:
                    # tail path: replicate sums across partitions on the PE
                    # (psum is idle by now), skip the DRAM round trip
                    rep_ps = s_pool.tile([H, QB], F32, name="rep_ps", tag="s_t")
                    nc.tensor.matmul(
                        rep_ps[:], ones_h[:], s_row[:], start=True, stop=True
                    )
                    s_rep = rep_ps
                else:
                    nc.sync.dma_start(out=sums[qb : qb + 1, :], in_=s_row[:])
                    s_rep = npool.tile([H, QB], F32, name="s_rep")
                    nc.sync.dma_start(
                        out=s_rep[:],
                        in_=bass.AP(
                            tensor=sums.tensor, offset=qb * QB,
                            ap=[[0, H], [1, QB]],
                        ),
                    )
                r_rep = npool.tile([H, QB], F32, name="r_rep")
                nc.vector.reciprocal_approx_fast(out=r_rep[:], in_=s_rep[:])
                o_n = npool.tile([H, QB], F32, name="o_n")
                nc.vector.tensor_mul(o_n[:], o_t[0:H, :], r_rep[:])
                nc.sync.dma_start(
                    out=outT[:, qb * QB : (qb + 1) * QB], in_=o_n[:]
                )

            def emit_available(nb, qbs):
                avail_pairs = 2 * (nb + 1)
                progress = True
                while progress:
                    progress = False
                    for qb in qbs:
                        if nb < qb:  # qb's queries come from x block qb
                            continue
                        if next_chunk[qb] < min(avail_pairs, NGROUPS):
                            emit_group(qb, next_chunk[qb])
                            next_chunk[qb] += 1
                            progress = True

            # --- production: x stream -> projections + qb0/1/2 attention ---
            x_tiles = {}

            def load_block(nb):
                if nb == 0:
                    x_tiles[0] = x_t0
                    return
                x_t = xpool.tile([128, ECH, QB], BF16, name="x_t")
                nc.sync.dma_start(
                    out=x_t[:], in_=xT[:, :, nb * QB : (nb + 1) * QB]
                )
                x_tiles[nb] = x_t

            load_block(0)
            load_block(1)
            load_block(2)
            kvT4 = kvT_sb.rearrange("p (c t) -> p c t", t=128)
            qT4 = qT_sb.rearrange("p (c t) -> p c t", t=128)
            for nb in range(NB):
                if nb + 3 < NB:
                    load_block(nb + 3)
                x_t = x_tiles.pop(nb)
                want_q = nb < QBLKS
                # x viewed as [128, ec, 4 chunks, 128]; per parity one M=128
                # stationary computes k AND v in one stream:
                #   even subchunks, [Wk|Wv]: psum[0:64]=k, psum[64:128]=v
                #   odd  subchunks, [Wv|Wk]: psum[0:64]=v, psum[64:128]=k
                x4 = x_t.rearrange("p e (c t) -> p e c t", t=128)
                kv_ps = kv_pool.tile([128, QB], F32, name="kv_ps", tag="kv")
                for ec in range(ECH):
                    nc.tensor.matmul(
                        kv_ps[:, 0:256], wsb[:, ec, 0:128],
                        x4[:, ec, 0:4:2, :],
                        start=(ec == 0), stop=(ec == ECH - 1),
                    )
                cb = nb * 4
                # drain evens while the odd chain runs
                nc.vector.tensor_copy(
                    kvT4[:, cb : cb + 4 : 2, :],
                    kv_ps[:, 0:256].rearrange("p (c t) -> p c t", t=128),
                )
                for ec in range(ECH):
                    nc.tensor.matmul(
                        kv_ps[:, 256:512], wsb[:, ec, 128:256],
                        x4[:, ec, 1:4:2, :],
                        start=(ec == 0), stop=(ec == ECH - 1),
                    )
                nc.vector.tensor_copy(
                    kvT4[:, cb + 1 : cb + 4 : 2, :],
                    kv_ps[:, 256:512].rearrange("p (c t) -> p c t", t=128),
                )
                if want_q:
                    # [Wq|Wq]: q lands in BOTH psum halves - the S row groups
                    # read the full query block from each half with no
                    # cross-duplication step
                    q_ps = qtr_pool.tile([128, QB], F32, name="q_ps", tag="qtr")
                    for ec in range(ECH):
                        nc.tensor.matmul(
                            q_ps[:, 0:256], wsb[:, ec, 256:384],
                            x4[:, ec, 0:4:2, :],
                            start=(ec == 0), stop=(ec == ECH - 1),
                        )
                    qcb = nb * 4
                    nc.vector.tensor_copy(
                        qT4[:, qcb : qcb + 4 : 2, :],
                        q_ps[:, 0:256].rearrange("p (c t) -> p c t", t=128),
                    )
                    for ec in range(ECH):
                        nc.tensor.matmul(
                            q_ps[:, 256:512], wsb[:, ec, 256:384],
                            x4[:, ec, 1:4:2, :],
                            start=(ec == 0), stop=(ec == ECH - 1),
                        )
                    nc.vector.tensor_copy(
                        qT4[:, qcb + 1 : qcb + 4 : 2, :],
                        q_ps[:, 256:512].rearrange("p (c t) -> p c t", t=128),
                    )
                # V-natural tiles via PE transpose (half follows chunk parity:
                # even chunks' v sits hi, odd chunks' v sits lo)
                for j in range(QB // 128):
                    c = nb * (QB // 128) + j
                    vlo = (j % 2) == 1
                    r = slice(0, H) if vlo else slice(H, 2 * H)
                    v_tr = qtr_pool.tile([128, H], BF16, name="v_tr", tag="qtr")
                    nc.tensor.transpose(
                        v_tr[:],
                        kvT_sb[r, (cb + j) * 128 : (cb + j + 1) * 128],
                        ident[r, :],
                        tile_position=(0 if vlo else H, 0),
                    )
                    nc.vector.tensor_copy(v_all[:, c, 0:H], v_tr[:])
                emit_available(nb, (0, 1))

            # --- drain: ALL FOUR qbs live at once. qb2/qb3's O accumulators
            # take over the kv and q/transpose banks (idle once the x stream
            # ends), so there are no phase-transition pipeline drains.
            # Weighted round-robin finishes qb0/qb1 early; their normalize
            # chains then hide under qb2/qb3's remaining groups.
            o_acc[2] = qtr_pool.tile([H + 1, QB], F32, name="o_qb2", tag="qtr")
            o_acc[3] = kv_pool.tile([H + 1, QB], F32, name="o_qb3", tag="kv")
            order = [0, 1, 2, 0, 1, 3]
            while any(next_chunk[qb] < NGROUPS for qb in range(QBLKS)):
                for qb in order:
                    if next_chunk[qb] < NGROUPS:
                        emit_group(qb, next_chunk[qb], phase=1)
                        next_chunk[qb] += 1
                for qb in (0, 1):
                    if next_chunk[qb] == NGROUPS and pv_lag[qb]:
                        finish_qb(qb)
            finish_qb(2)
            finish_qb(3, fast=True)

    nc.compile()
    return nc


_NC_CACHE = {}


def _get_nc():
    if "nc" not in _NC_CACHE:
        _NC_CACHE["nc"] = build_kernel()
    return _NC_CACHE["nc"]


def _make_in_maps(x, Wk, Wq, Wv):
    BF = ml_dtypes.bfloat16
    wkT, wqT, wvT = Wk.T, Wq.T, Wv.T  # [E, H] each
    wpack = np.ascontiguousarray(
        np.concatenate([wkT, wvT, wvT, wkT, wqT, wqT], axis=1), dtype=np.float32
    ).astype(BF)
    in_maps = []
    for c in range(NCORES):
        b, h = divmod(c, 2)
        xb = np.asarray(x[b], dtype=np.float32)
        if h == 1:
            xb = np.concatenate([xb[NQ:], xb[:NQ]], axis=0)
        in_maps.append(
            {"xT": np.ascontiguousarray(xb.T).astype(BF), "wT": wpack}
        )
    return in_maps


def kernel(x, Wk, Wq, Wv, _trace=False, _tmpdir=None):
    nc = _get_nc()
    in_maps = _make_in_maps(x, Wk, Wq, Wv)
    kwargs = {}
    if _trace:
        kwargs = dict(trace=True, tmpdir=_tmpdir or tempfile.mkdtemp())
    res = run_bass_kernel_spmd(nc, in_maps, core_ids=list(range(NCORES)), **kwargs)
    out = np.empty((B, N, H), np.float32)
    for c in range(NCORES):
        b, h = divmod(c, 2)
        out[b, h * NQ : (h + 1) * NQ, :] = res.results[c]["outT"].T
    if _trace:
        return out, res
    return out
